# revision 1
# baseline (speedup 1.0000x reference)
"""Trainium2 Bass kernel for nn_KS_8134668058856 (histogram_binning KS statistic).

Strategy (data-parallel over 8 NeuronCores):
  - Each core streams its shard of preds/targets, computes
    u = 2 * round(10000 * sigmoid(x)) + target   in [0, 20002)
    (round-to-nearest matches XLA-on-neuron's f32->s32 convert of the
    reference; sigmoid via the ACT engine).
  - 2-level histogram: u = coarse*128 + fine.  Per chunk of 128 elements,
    build a fine one-hot [128p, 128] and a coarse one-hot [128p, 157] with
    DVE is_equal against static iota tiles, then accumulate
    psum[fine, coarse] += fineOH^T @ coarseOH on the PE (PSUM f32, exact
    integer counts).
  - One-hot layout: chunk-PAIR interleaved ("p (g j k)" with k=2 pairs
    innermost).  All is_equal operands are 16-bit with unit innermost
    stride -> DVE runs in 2x_1P mode (2 elem/cycle/lane); the matmul
    operand slices then have 4-byte column strides, which the PE streams
    at full rate (71 ns per 128x128x157 matmul, LDWEIGHTS hidden).
  - Host: sum the 8 per-core 2-D histograms, unpack to tp/fp (u odd/even),
    then replicate the reference tail (f32 cumsum -> normalize -> max |diff|)
    with jnp on the default backend.
"""
import sys

sys.path.insert(0, "/opt/trn_rl_repo")

import numpy as np

import concourse.bacc as bacc
import concourse.mybir as mybir
import concourse.tile as tile
from concourse.bass_utils import run_bass_kernel_spmd

M = mybir
P = 128            # partitions / fine bins
NC = 8             # cores
NBINS = 10001
NU = 2 * NBINS     # combined index range: u = 2*bin + target
C_W = 157          # coarse bins: ceil(20002 / 128)
TWO23 = 8388608.0  # 2^23 for round-to-nearest-even trick

_CACHE = {}


def build_nc(n_tiles: int, F: int, G: int = 32):
    """Build the per-core SPMD kernel.

    Processes n_tiles * P * F elements per core.  F/2 chunk-pairs per tile;
    G pairs of one-hots are built per DVE instruction.
    """
    assert F % (2 * G) == 0
    cols = n_tiles * F
    nc = bacc.Bacc(None)
    preds = nc.declare_dram_parameter("preds", [P, cols], M.dt.float32, isOutput=False)
    targets = nc.declare_dram_parameter("targets", [P, cols], M.dt.float32, isOutput=False)
    # pair-replicated iota tiles: value[p, j*2 + k] = j  (bf16)
    iota_f = nc.declare_dram_parameter("iota_f", [P, P * 2], M.dt.bfloat16, isOutput=False)
    iota_c = nc.declare_dram_parameter("iota_c", [P, C_W * 2], M.dt.bfloat16, isOutput=False)
    hist = nc.declare_dram_parameter("hist", [P, C_W], M.dt.float32, isOutput=True)

    n_chunks_total = n_tiles * F
    N_ACC = 4
    n_grp = F // (2 * G)

    # const APs for ACT activation biases
    for val in (TWO23, -TWO23, -0.49951171875):
        t = nc.alloc_sbuf_tensor(f"const-float32-{val}", [128, 1], M.dt.float32)
        nc.gpsimd.memset(t.ap(), val)
        nc.const_aps.aps[(M.dt.float32, val)] = t.ap()
    nc.all_engine_barrier()

    with tile.TileContext(nc) as tc:
        with (
            tc.tile_pool(name="consts", bufs=1) as cpool,
            tc.tile_pool(name="io", bufs=4) as iopool,
            tc.tile_pool(name="work", bufs=3) as wpool,
            tc.tile_pool(name="oh", bufs=2) as ohpool,
            tc.tile_pool(name="psum", bufs=1, space="PSUM") as ppool,
            tc.tile_pool(name="outp", bufs=1) as opool,
        ):
            iota_f_t = cpool.tile([P, P * 2], M.dt.bfloat16, tag="iota_f")
            iota_c_t = cpool.tile([P, C_W * 2], M.dt.bfloat16, tag="iota_c")
            nc.sync.dma_start(out=iota_f_t[:], in_=iota_f[:])
            nc.sync.dma_start(out=iota_c_t[:], in_=iota_c[:])
            iota_f_4d = iota_f_t[:].rearrange("p (j k) -> p j k", k=2)
            iota_c_4d = iota_c_t[:].rearrange("p (j k) -> p j k", k=2)

            accs = [ppool.tile([P, C_W], M.dt.float32, name=f"acc{a}", tag=f"acc{a}")
                    for a in range(N_ACC)]

            gk = 0  # global chunk counter
            for ti in range(n_tiles):
                sl = slice(ti * F, (ti + 1) * F)
                xt = iopool.tile([P, F], M.dt.float32, tag="xt")
                tt = iopool.tile([P, F], M.dt.float32, tag="tt")
                nc.sync.dma_start(out=xt[:], in_=preds[:, sl])
                nc.sync.dma_start(out=tt[:], in_=targets[:, sl])

                st = wpool.tile([P, F], M.dt.float32, tag="st")
                nc.scalar.activation(st[:], xt[:], M.ActivationFunctionType.Sigmoid)

                # rb = rint(10000*sigmoid) via 2^23 round trip
                # NOTE: must stay on DVE tensor_scalar — the two ALU stages
                # round the *1e4 product to f32 before adding 2^23, matching
                # the reference's separate mul+convert.  ACT Identity fuses
                # scale*x+bias and flips ~0.03% of bins.
                t1 = wpool.tile([P, F], M.dt.float32, tag="tB")
                nc.vector.tensor_scalar(
                    t1[:], st[:], 10000.0, scalar2=TWO23,
                    op0=M.AluOpType.mult, op1=M.AluOpType.add,
                )
                rb = wpool.tile([P, F], M.dt.float32, tag="rbA")
                nc.scalar.activation(
                    rb[:], t1[:], M.ActivationFunctionType.Identity,
                    bias=-TWO23, scale=1.0,
                )
                # u = 2*rb + target
                ut = wpool.tile([P, F], M.dt.float32, tag="ut")
                nc.vector.scalar_tensor_tensor(
                    out=ut[:], in0=rb[:], scalar=2.0, in1=tt[:],
                    op0=M.AluOpType.mult, op1=M.AluOpType.add,
                )
                # coarse = floor(u/128) = rint(u/128 - (0.5 - 2^-11)).
                # u/128 has fraction k/128 exactly; shifting by just under 0.5
                # keeps every value strictly inside (k-0.5, k+0.5) so rint
                # floors correctly (plain -0.5 ties to even at fraction 0).
                # Runs on the (otherwise idle) ACT engine: Identity(x*s + b);
                # the ~5e-4 slack makes any fused-rounding difference harmless.
                c1 = wpool.tile([P, F], M.dt.float32, tag="tA")
                nc.scalar.activation(
                    c1[:], ut[:], M.ActivationFunctionType.Identity,
                    bias=-0.49951171875, scale=0.0078125,
                )
                ct2 = wpool.tile([P, F], M.dt.float32, tag="tB")
                nc.scalar.activation(
                    ct2[:], c1[:], M.ActivationFunctionType.Identity,
                    bias=TWO23, scale=1.0,
                )
                ct = wpool.tile([P, F], M.dt.float32, tag="tA")
                nc.scalar.activation(
                    ct[:], ct2[:], M.ActivationFunctionType.Identity,
                    bias=-TWO23, scale=1.0,
                )
                # fine = u - 128*coarse
                ft = wpool.tile([P, F], M.dt.float32, tag="ft")
                nc.vector.scalar_tensor_tensor(
                    out=ft[:], in0=ct[:], scalar=-128.0, in1=ut[:],
                    op0=M.AluOpType.mult, op1=M.AluOpType.add,
                )
                # bf16 copies of fine/coarse (values < 256 are exact in bf16);
                # casts run on ACT (activation Copy) to spare the DVE.
                ft_bf = wpool.tile([P, F], M.dt.bfloat16, tag="ft_bf")
                ct_bf = wpool.tile([P, F], M.dt.bfloat16, tag="ct_bf")
                nc.scalar.copy(out=ft_bf[:], in_=ft[:])
                nc.scalar.copy(out=ct_bf[:], in_=ct[:])
                ft_pairs = ft_bf[:].rearrange("p (g k) -> p g k", k=2)
                ct_pairs = ct_bf[:].rearrange("p (g k) -> p g k", k=2)

                for grp in range(n_grp):
                    gs = slice(grp * G, (grp + 1) * G)
                    # pair-interleaved one-hots: oh[p, g, j, k] = (val[p, 2(g0+g)+k] == j)
                    f_oh = ohpool.tile([P, G * P * 2], M.dt.bfloat16, tag="f_oh")
                    c_oh = ohpool.tile([P, G * C_W * 2], M.dt.bfloat16, tag="c_oh")
                    nc.vector.tensor_tensor(
                        out=f_oh[:].rearrange("p (g j k) -> p g j k", j=P, k=2),
                        in0=ft_pairs[:, gs, None, :].broadcast_to([P, G, P, 2]),
                        in1=iota_f_4d[:, None, :, :].broadcast_to([P, G, P, 2]),
                        op=M.AluOpType.is_equal,
                    )
                    nc.vector.tensor_tensor(
                        out=c_oh[:].rearrange("p (g j k) -> p g j k", j=C_W, k=2),
                        in0=ct_pairs[:, gs, None, :].broadcast_to([P, G, C_W, 2]),
                        in1=iota_c_4d[:, None, :, :].broadcast_to([P, G, C_W, 2]),
                        op=M.AluOpType.is_equal,
                    )
                    f_mm = f_oh[:].rearrange("p (g j k) -> p g k j", j=P, k=2)
                    c_mm = c_oh[:].rearrange("p (g j k) -> p g k j", j=C_W, k=2)
                    for q in range(G):
                        for kp in range(2):
                            acc = accs[gk % N_ACC]
                            nc.tensor.matmul(
                                acc[:],
                                f_mm[:, q, kp, :],
                                c_mm[:, q, kp, :],
                                start=(gk < N_ACC),
                                stop=(gk >= n_chunks_total - N_ACC),
                            )
                            gk += 1

            # merge the 4 accumulators (PSUM -> SBUF copies, then adds) and write out
            hs = []
            for a in range(N_ACC):
                h = opool.tile([P, C_W], M.dt.float32, name=f"h{a}", tag=f"h{a}")
                nc.vector.tensor_copy(out=h[:], in_=accs[a][:])
                hs.append(h)
            nc.vector.tensor_tensor(out=hs[0][:], in0=hs[0][:], in1=hs[1][:], op=M.AluOpType.add)
            nc.vector.tensor_tensor(out=hs[2][:], in0=hs[2][:], in1=hs[3][:], op=M.AluOpType.add)
            nc.vector.tensor_tensor(out=hs[0][:], in0=hs[0][:], in1=hs[2][:], op=M.AluOpType.add)
            nc.sync.dma_start(out=hist[:], in_=hs[0][:])

    nc.finalize()
    return nc


def _get_nc(n_tiles: int, F: int, G: int):
    key = (n_tiles, F, G)
    if key not in _CACHE:
        _CACHE[key] = build_nc(n_tiles, F, G)
    return _CACHE[key]


def _iota_tiles():
    import ml_dtypes
    jf = np.repeat(np.arange(P, dtype=np.float32), 2)          # [P*2] pair-replicated
    jc = np.repeat(np.arange(C_W, dtype=np.float32), 2)        # [C_W*2]
    iota_f = np.broadcast_to(jf, (P, P * 2)).astype(ml_dtypes.bfloat16)
    iota_c = np.broadcast_to(jc, (P, C_W * 2)).astype(ml_dtypes.bfloat16)
    return np.ascontiguousarray(iota_f), np.ascontiguousarray(iota_c)


def run_hist(preds: np.ndarray, targets: np.ndarray, n_tiles: int, F: int, G: int = 32):
    """Run the SPMD kernel; returns summed histogram over combined index u (float64[NU])."""
    nc = _get_nc(n_tiles, F, G)
    cols = n_tiles * F
    per_core = P * cols
    assert preds.size == NC * per_core, (preds.size, NC * per_core)
    ps = np.ascontiguousarray(preds.reshape(NC, P, cols), dtype=np.float32)
    ts = np.ascontiguousarray(targets.reshape(NC, P, cols), dtype=np.float32)
    iota_f, iota_c = _iota_tiles()
    in_maps = [
        {"preds": ps[c], "targets": ts[c], "iota_f": iota_f, "iota_c": iota_c}
        for c in range(NC)
    ]
    res = run_bass_kernel_spmd(nc, in_maps, core_ids=list(range(NC)))
    h2d = np.zeros((P, C_W), dtype=np.float64)
    for c in range(NC):
        h2d += res.results[c]["hist"].astype(np.float64)
    # unpack: u = coarse*128 + fine  ->  hist2d[fine, coarse]
    hist_u = h2d.T.reshape(-1)[:NU]  # [coarse, fine] flattened = u-major
    return hist_u


def kernel(preds: np.ndarray, targets: np.ndarray) -> np.ndarray:
    preds = np.asarray(preds, dtype=np.float32).reshape(-1)
    targets = np.asarray(targets, dtype=np.float32).reshape(-1)
    N = preds.size
    assert N == 33554432, N
    # full size: per core 4,194,304 = 128 * 32768 ; 32 tiles of F=1024
    hist_u = run_hist(preds, targets, n_tiles=32, F=1024, G=32)

    fp = hist_u[0::2].astype(np.float32)  # target == 0
    tp = hist_u[1::2].astype(np.float32)  # target == 1

    # replicate the reference tail in f32 on the default jax backend
    # (matches the reference's cumsum rounding bit-for-bit); fall back to
    # numpy f32 if jax is unavailable.
    try:
        import jax.numpy as jnp

        tp_cum = jnp.cumsum(jnp.asarray(tp))
        fp_cum = jnp.cumsum(jnp.asarray(fp))
        tp_curve = tp_cum / tp_cum[-1]
        fp_curve = fp_cum / fp_cum[-1]
        out = jnp.max(jnp.abs(tp_curve - fp_curve))
        return np.asarray(out)
    except Exception:
        tp_cum = np.cumsum(tp, dtype=np.float32)
        fp_cum = np.cumsum(fp, dtype=np.float32)
        tp_curve = (tp_cum / tp_cum[-1]).astype(np.float32)
        fp_curve = (fp_cum / fp_cum[-1]).astype(np.float32)
        return np.float32(np.max(np.abs(tp_curve - fp_curve)))



# revision 4
# speedup vs baseline: 1.3495x; 1.3495x over previous
"""Trainium2 Bass kernel for nn_KS_8134668058856 (histogram_binning KS statistic).

Strategy (data-parallel over 8 NeuronCores):
  - HOST: partition elements by target (order-invariant for histograms),
    pad each part to a multiple of 8*8192, shard both parts across cores.
    Each 128-element chunk is then single-target, so the kernel bins
    bin = rint(10000*sigmoid(x)) in [0, 10001) directly:
      fine = bin mod 128 (128 one-hot slots), coarse = bin div 128 (79 slots)
    = 207 DVE one-hot slots/element vs 285 for the mixed-target encoding.
  - 2-level histogram per chunk: fine one-hot [128p, 128] and coarse one-hot
    [128p, 79] built with DVE is_equal against static iota tiles (bf16
    pair-interleaved -> 2x_1P DVE mode), accumulated with
    psum[fine, coarse] += fineOH^T @ coarseOH on the PE.  Groups of chunks
    before the target boundary accumulate into the tp psum set, after it
    into the fp set (boundary is a compile-time constant derived from the
    runtime target counts; the bass kernel is built per run).
  - Host: sum per-core 2-D histograms, strip the padding counts,
    then replicate the reference tail (f32 cumsum -> normalize -> max |diff|).
"""
import sys

sys.path.insert(0, "/opt/trn_rl_repo")

import numpy as np

import concourse.bacc as bacc
import concourse.mybir as mybir
import concourse.tile as tile
from concourse.bass_utils import run_bass_kernel_spmd

M = mybir
P = 128            # partitions / fine bins
NC = 8             # cores
NBINS = 10001
C_W = 79           # coarse bins: ceil(10001 / 128)
TWO23 = 8388608.0  # 2^23 for round-to-nearest-even trick
GROUP_ELEMS = 8192  # one one-hot group: G=32 pairs = 64 chunks of 128
G = 32
PAD_PRED = 30.0    # sigmoid -> 1.0 -> bin 10000 exactly

_CACHE = {}


def build_nc(n_grp_tp: int, n_grp_fp: int):
    """Per-core SPMD kernel: n_grp_tp one-hot groups accumulate into the tp
    histogram, the following n_grp_fp groups into the fp histogram.  Each
    group is G=32 chunk-pairs = 64 chunks = 8192 elements."""
    n_grp_total = n_grp_tp + n_grp_fp
    GRP_TILE = 16          # groups per DMA/prep tile (F = 1024 cols)
    cols_total = n_grp_total * 2 * G
    nc = bacc.Bacc(None)
    preds = nc.declare_dram_parameter("preds", [P, cols_total], M.dt.float32, isOutput=False)
    iota_f = nc.declare_dram_parameter("iota_f", [P, P * 2], M.dt.bfloat16, isOutput=False)
    iota_c = nc.declare_dram_parameter("iota_c", [P, C_W * 2], M.dt.bfloat16, isOutput=False)
    hist_tp = nc.declare_dram_parameter("hist_tp", [P, C_W], M.dt.float32, isOutput=True)
    hist_fp = nc.declare_dram_parameter("hist_fp", [P, C_W], M.dt.float32, isOutput=True)

    N_ACC = 4

    # const APs for ACT activation biases
    for val in (TWO23, -TWO23, -0.49951171875):
        t = nc.alloc_sbuf_tensor(f"const-float32-{val}", [128, 1], M.dt.float32)
        nc.gpsimd.memset(t.ap(), val)
        nc.const_aps.aps[(M.dt.float32, val)] = t.ap()
    nc.all_engine_barrier()

    # tile boundaries: tiles of up to GRP_TILE groups
    tiles = []  # (col_start, n_grp_this_tile)
    g = 0
    while g < n_grp_total:
        ng = min(GRP_TILE, n_grp_total - g)
        tiles.append((g, ng))
        g += ng

    with tile.TileContext(nc) as tc:
        with (
            tc.tile_pool(name="consts", bufs=1) as cpool,
            tc.tile_pool(name="io", bufs=4) as iopool,
            tc.tile_pool(name="work", bufs=3) as wpool,
            tc.tile_pool(name="oh", bufs=2) as ohpool,
            tc.tile_pool(name="psum", bufs=1, space="PSUM") as ppool,
            tc.tile_pool(name="outp", bufs=1) as opool,
        ):
            iota_f_t = cpool.tile([P, P * 2], M.dt.bfloat16, tag="iota_f")
            iota_c_t = cpool.tile([P, C_W * 2], M.dt.bfloat16, tag="iota_c")
            nc.sync.dma_start(out=iota_f_t[:], in_=iota_f[:])
            nc.sync.dma_start(out=iota_c_t[:], in_=iota_c[:])
            iota_f_4d = iota_f_t[:].rearrange("p (j k) -> p j k", k=2)
            iota_c_4d = iota_c_t[:].rearrange("p (j k) -> p j k", k=2)

            accs_tp = [ppool.tile([P, C_W], M.dt.float32, name=f"acct{a}", tag=f"acct{a}")
                       for a in range(N_ACC)]
            accs_fp = [ppool.tile([P, C_W], M.dt.float32, name=f"accf{a}", tag=f"accf{a}")
                       for a in range(N_ACC)]

            n_chunks_tp = n_grp_tp * 2 * G
            n_chunks_fp = n_grp_fp * 2 * G
            gk_tp = 0  # chunk counters per segment
            gk_fp = 0
            for (g0, ng) in tiles:
                F = ng * 2 * G
                sl = slice(g0 * 2 * G, g0 * 2 * G + F)
                xt = iopool.tile([P, F], M.dt.float32, tag="xt")
                nc.sync.dma_start(out=xt[:], in_=preds[:, sl])

                st = wpool.tile([P, F], M.dt.float32, tag="st")
                nc.scalar.activation(st[:], xt[:], M.ActivationFunctionType.Sigmoid)

                # rb = rint(10000*sigmoid) via 2^23 round trip
                # NOTE: must stay on DVE tensor_scalar — the two ALU stages
                # round the *1e4 product to f32 before adding 2^23, matching
                # the reference's separate mul+convert.
                t1 = wpool.tile([P, F], M.dt.float32, tag="tB")
                nc.vector.tensor_scalar(
                    t1[:], st[:], 10000.0, scalar2=TWO23,
                    op0=M.AluOpType.mult, op1=M.AluOpType.add,
                )
                ut = wpool.tile([P, F], M.dt.float32, tag="ut")
                nc.scalar.activation(
                    ut[:], t1[:], M.ActivationFunctionType.Identity,
                    bias=-TWO23, scale=1.0,
                )
                # coarse = floor(bin/128) = rint(bin/128 - (0.5 - 2^-11));
                # bin/128 has fraction k/128 exactly, the shift keeps every
                # value strictly inside (c-0.5, c+0.5) so rint floors.
                c1 = wpool.tile([P, F], M.dt.float32, tag="tA")
                nc.scalar.activation(
                    c1[:], ut[:], M.ActivationFunctionType.Identity,
                    bias=-0.49951171875, scale=0.0078125,
                )
                ct2 = wpool.tile([P, F], M.dt.float32, tag="tB")
                nc.scalar.activation(
                    ct2[:], c1[:], M.ActivationFunctionType.Identity,
                    bias=TWO23, scale=1.0,
                )
                ct = wpool.tile([P, F], M.dt.float32, tag="tA")
                nc.scalar.activation(
                    ct[:], ct2[:], M.ActivationFunctionType.Identity,
                    bias=-TWO23, scale=1.0,
                )
                # fine = bin - 128*coarse
                ft = wpool.tile([P, F], M.dt.float32, tag="ft")
                nc.vector.scalar_tensor_tensor(
                    out=ft[:], in0=ct[:], scalar=-128.0, in1=ut[:],
                    op0=M.AluOpType.mult, op1=M.AluOpType.add,
                )
                # bf16 copies (values < 256 exact); casts on ACT
                ft_bf = wpool.tile([P, F], M.dt.bfloat16, tag="ft_bf")
                ct_bf = wpool.tile([P, F], M.dt.bfloat16, tag="ct_bf")
                nc.scalar.copy(out=ft_bf[:], in_=ft[:])
                nc.scalar.copy(out=ct_bf[:], in_=ct[:])
                ft_pairs = ft_bf[:].rearrange("p (g k) -> p g k", k=2)
                ct_pairs = ct_bf[:].rearrange("p (g k) -> p g k", k=2)

                for grp in range(ng):
                    grp_global = g0 + grp
                    is_tp = grp_global < n_grp_tp
                    gs = slice(grp * G, (grp + 1) * G)
                    f_oh = ohpool.tile([P, G * P * 2], M.dt.bfloat16, tag="f_oh")
                    c_oh = ohpool.tile([P, G * C_W * 2], M.dt.bfloat16, tag="c_oh")
                    nc.vector.tensor_tensor(
                        out=f_oh[:].rearrange("p (g j k) -> p g j k", j=P, k=2),
                        in0=ft_pairs[:, gs, None, :].broadcast_to([P, G, P, 2]),
                        in1=iota_f_4d[:, None, :, :].broadcast_to([P, G, P, 2]),
                        op=M.AluOpType.is_equal,
                    )
                    nc.vector.tensor_tensor(
                        out=c_oh[:].rearrange("p (g j k) -> p g j k", j=C_W, k=2),
                        in0=ct_pairs[:, gs, None, :].broadcast_to([P, G, C_W, 2]),
                        in1=iota_c_4d[:, None, :, :].broadcast_to([P, G, C_W, 2]),
                        op=M.AluOpType.is_equal,
                    )
                    f_mm = f_oh[:].rearrange("p (g j k) -> p g k j", j=P, k=2)
                    c_mm = c_oh[:].rearrange("p (g j k) -> p g k j", j=C_W, k=2)
                    for q in range(G):
                        for kp in range(2):
                            if is_tp:
                                acc = accs_tp[gk_tp % N_ACC]
                                start = gk_tp < N_ACC
                                stop = gk_tp >= n_chunks_tp - N_ACC
                                gk_tp += 1
                            else:
                                acc = accs_fp[gk_fp % N_ACC]
                                start = gk_fp < N_ACC
                                stop = gk_fp >= n_chunks_fp - N_ACC
                                gk_fp += 1
                            nc.tensor.matmul(
                                acc[:],
                                f_mm[:, q, kp, :],
                                c_mm[:, q, kp, :],
                                start=start,
                                stop=stop,
                            )

            # merge the accumulators and write out
            for accs, hist in ((accs_tp, hist_tp), (accs_fp, hist_fp)):
                hs = []
                for a in range(N_ACC):
                    h = opool.tile([P, C_W], M.dt.float32,
                                   name=f"h{hist.name}{a}", tag=f"h{hist.name}{a}")
                    nc.vector.tensor_copy(out=h[:], in_=accs[a][:])
                    hs.append(h)
                nc.vector.tensor_tensor(out=hs[0][:], in0=hs[0][:], in1=hs[1][:], op=M.AluOpType.add)
                nc.vector.tensor_tensor(out=hs[2][:], in0=hs[2][:], in1=hs[3][:], op=M.AluOpType.add)
                nc.vector.tensor_tensor(out=hs[0][:], in0=hs[0][:], in1=hs[2][:], op=M.AluOpType.add)
                nc.sync.dma_start(out=hist[:], in_=hs[0][:])

    nc.finalize()
    return nc


def _get_nc(n_grp_tp: int, n_grp_fp: int):
    key = (n_grp_tp, n_grp_fp)
    if key not in _CACHE:
        _CACHE[key] = build_nc(n_grp_tp, n_grp_fp)
    return _CACHE[key]


def _iota_tiles():
    import ml_dtypes
    jf = np.repeat(np.arange(P, dtype=np.float32), 2)
    jc = np.repeat(np.arange(C_W, dtype=np.float32), 2)
    iota_f = np.broadcast_to(jf, (P, P * 2)).astype(ml_dtypes.bfloat16)
    iota_c = np.broadcast_to(jc, (P, C_W * 2)).astype(ml_dtypes.bfloat16)
    return np.ascontiguousarray(iota_f), np.ascontiguousarray(iota_c)


def _pad_part(x: np.ndarray):
    """Pad a 1-D part to a multiple of NC*GROUP_ELEMS with PAD_PRED."""
    q = NC * GROUP_ELEMS
    n_pad = (-x.size) % q
    if n_pad:
        x = np.concatenate([x, np.full(n_pad, PAD_PRED, dtype=np.float32)])
    return x, n_pad


def _prepare(preds: np.ndarray, targets: np.ndarray):
    """Partition by target, pad, shard; returns (nc, in_maps, tp_pad, fp_pad)."""
    mask = targets >= 0.5
    tp_part, tp_pad = _pad_part(np.ascontiguousarray(preds[mask], dtype=np.float32))
    fp_part, fp_pad = _pad_part(np.ascontiguousarray(preds[~mask], dtype=np.float32))
    n_grp_tp = tp_part.size // (NC * GROUP_ELEMS)
    n_grp_fp = fp_part.size // (NC * GROUP_ELEMS)
    nc = _get_nc(n_grp_tp, n_grp_fp)

    # shard: per core, tp groups then fp groups, laid out [P, cols] per core
    # (any fixed element order works for a histogram).
    tp3 = tp_part.reshape(NC, P, -1)
    fp3 = fp_part.reshape(NC, P, -1)
    iota_f, iota_c = _iota_tiles()
    in_maps = []
    for c in range(NC):
        pc = np.concatenate([tp3[c], fp3[c]], axis=1)
        in_maps.append({"preds": np.ascontiguousarray(pc),
                        "iota_f": iota_f, "iota_c": iota_c})
    return nc, in_maps, tp_pad, fp_pad


def run_hist(preds: np.ndarray, targets: np.ndarray):
    """Returns (hist_tp, hist_fp) as float64[NBINS] (padding removed)."""
    nc, in_maps, tp_pad, fp_pad = _prepare(preds, targets)
    res = run_bass_kernel_spmd(nc, in_maps, core_ids=list(range(NC)))
    h_tp = np.zeros((P, C_W), dtype=np.float64)
    h_fp = np.zeros((P, C_W), dtype=np.float64)
    for c in range(NC):
        h_tp += res.results[c]["hist_tp"].astype(np.float64)
        h_fp += res.results[c]["hist_fp"].astype(np.float64)
    # [fine, coarse] -> bin-major flatten: bin = coarse*128 + fine
    hist_tp = h_tp.T.reshape(-1)[:NBINS].copy()
    hist_fp = h_fp.T.reshape(-1)[:NBINS].copy()
    # padding went to bin 10000 exactly
    hist_tp[10000] -= tp_pad
    hist_fp[10000] -= fp_pad
    return hist_tp, hist_fp


def kernel(preds: np.ndarray, targets: np.ndarray) -> np.ndarray:
    preds = np.asarray(preds, dtype=np.float32).reshape(-1)
    targets = np.asarray(targets, dtype=np.float32).reshape(-1)
    tp, fp = run_hist(preds, targets)
    tp = tp.astype(np.float32)
    fp = fp.astype(np.float32)

    # replicate the reference tail in f32 on the default jax backend
    try:
        import jax.numpy as jnp

        tp_cum = jnp.cumsum(jnp.asarray(tp))
        fp_cum = jnp.cumsum(jnp.asarray(fp))
        tp_curve = tp_cum / tp_cum[-1]
        fp_curve = fp_cum / fp_cum[-1]
        out = jnp.max(jnp.abs(tp_curve - fp_curve))
        return np.asarray(out)
    except Exception:
        tp_cum = np.cumsum(tp, dtype=np.float32)
        fp_cum = np.cumsum(fp, dtype=np.float32)
        tp_curve = (tp_cum / tp_cum[-1]).astype(np.float32)
        fp_curve = (fp_cum / fp_cum[-1]).astype(np.float32)
        return np.float32(np.max(np.abs(tp_curve - fp_curve)))
